# revision 6
# baseline (speedup 1.0000x reference)
"""Trainium2 Bass kernel: masked attention with softmax over the query axis (dim 1).

Reference computation (per batch b):
    q = x_q @ Wq.T + bq; k = x_k @ Wk.T + bk; v = x_v @ Wv.T + bv
    score = q @ k.T / sqrt(dk) + (-1e9 where mask==0)
    attn = softmax(score, axis=Sq)   # softmax over the QUERY axis
    y = attn @ v
Sharding: 8 cores = 4 batches x 2 Sk-halves. The softmax axis (Sq) stays whole on
every core; each core produces a partial y (sum over its Sk half) and the host
adds the two halves per batch.

v2 design (per core; measured v1 engine busy: PE 82us, Act 81, DVE 80, DMA 98):
  - qT [64, sq] / kT [64, sk] all live in partitions 0:63 (no row packing), so
    kT needs no duplicated copy: k-projection is 8 matmuls per 512 cols, not 16.
  - Act engine runs EXP ONLY (64 x [128,1024] PSUM tiles ~ 72us = the pacer).
    All PSUM->SBUF copies (q/k proj bias-adds, v scaling by 1/den, y output)
    run on DVE via tensor_scalar with per-partition AP scalars.
  - Mask multiply + den row-sum: one scalar_tensor_tensor per [128,1024]
    quarter-row, split between DVE and GpSimd (Pool) so neither exceeds the
    Act pace. den = X-reduce of the 4 quarter accumulators.
  - Phase A = two sweeps over si (q-blocks 0,1 then 2,3) so the first exp fires
    after only ~3MB of input DMA and x_q block deadlines are staggered.
  - Phase B per si: scores p=2,3, stt quarters 2,3, den reduce, recip,
    v(si-2) scaled by rec, y(si-3) accumulating into 4 persistent PSUM tiles.
  - DMA issue order == consumption deadline order on the single HWDGE queue.
Numerics: fp16 storage, f32 PSUM accumulation (v1 measured resid_var ~6e-7).
"""

import numpy as np

B, SQ, SK, D_MODEL, D_K = 4, 4096, 4096, 1024, 64
N_CORES = 8
SK_SHARD = SK // (N_CORES // B)  # 2048
V_LAG = 2
Y_LAG = 3

# stt quarter-tiles handled by GpSimd (Pool) instead of DVE, per sweep.
# si sets chosen so Pool tiles are spaced (avoids straggle on the in-order q).
POOL_A = frozenset((1, 3, 5, 7, 9, 11, 13, 15))   # sweeps 1+2: 8 of 16 each
POOL_B = frozenset((1, 3, 5, 7, 9, 11, 13, 15))   # phase B quarters: 8 of 16


def emit_kernel(tc, aps, sq, sk, d, dk):
    """Emit the per-core attention kernel into TileContext tc."""
    from contextlib import ExitStack

    from concourse import mybir

    nc = tc.nc
    f16 = mybir.dt.float16
    f32 = mybir.dt.float32
    u8 = mybir.dt.uint8
    AF = mybir.ActivationFunctionType
    ALU = mybir.AluOpType

    n_d = d // 128            # d_model chunks (8)
    n_si = sk // 128          # key chunks = partition dim of scoreT (16)
    n_qb = sq // 512          # q blocks of 512 (8)
    n_qp = n_qb // 2          # q pairs -> one [128,1024] exp tile (4)
    n_sj = sk // 512          # k projection column blocks (4)
    n_vq = max(n_si // 4, 1)  # si per x_v quarter (4)

    x_qP, x_kP, x_vP = aps["x_qP"], aps["x_kP"], aps["x_vP"]
    maskT = aps["maskT"]
    wall = aps["wall"]          # [128, 3, n_d, dk] f16: Wq/Wk/Wv d-chunks
    bias = aps["bias"]          # [128, 2+dk] f32: bq/8 | bk | (unused)
    bv16 = aps["bv16"]          # [1, dk] f16
    out = aps["out"]

    with ExitStack() as ctx:
        const = ctx.enter_context(tc.tile_pool(name="const", bufs=1))
        persist = ctx.enter_context(tc.tile_pool(name="persist", bufs=1))
        stat_p = ctx.enter_context(tc.tile_pool(name="statp", bufs=1))
        mask_p = ctx.enter_context(tc.tile_pool(name="maskp", bufs=8))
        attn_p = ctx.enter_context(tc.tile_pool(name="attnp", bufs=n_si))
        psA = ctx.enter_context(tc.tile_pool(name="psA", bufs=2, space="PSUM"))

        # ---------------- constants ----------------
        w_sb = const.tile([128, 3, n_d, dk], f16, name="w_sb")
        nc.sync.dma_start(w_sb[:], wall[:])
        b_sb = const.tile([128, 2 + dk], f32, name="b_sb")
        nc.sync.dma_start(b_sb[:], bias[:])
        bv_sb = const.tile([1, dk], f16, name="bv_sb")
        nc.sync.dma_start(bv_sb[:], bv16[:])
        ones_sb = const.tile([1, 128], f16, name="ones_sb")
        nc.vector.memset(ones_sb[:], 1.0)
        bq8 = b_sb[0:64, 0:1]   # bq / 8
        bk1 = b_sb[0:64, 1:2]   # bk

        qT = persist.tile([64, sq], f16, name="qT")
        kT = persist.tile([64, sk], f16, name="kT")
        vs_sb = persist.tile([128, n_si, dk], f16, name="vs_sb")
        den4 = stat_p.tile([128, n_si, 4], f32, name="den4")
        den = stat_p.tile([128, n_si], f32, name="den")
        rec = stat_p.tile([128, n_si], f32, name="rec")

        attn_t = []
        masks = [[None] * n_si for _ in range(4)]  # [q-quarter][si]

        def alloc_mask(blk, si):
            mt = mask_p.tile([128, 1024], u8, name="mask_t")
            nc.sync.dma_start(
                mt[:], maskT[si * 128:(si + 1) * 128,
                             blk * 1024:(blk + 1) * 1024])
            masks[blk][si] = mt

        def stt_part(si, blk):
            # masked attn + quarter-row den accumulation. Pool can't run
            # TensorScalarPtr (backend engine check), so Pool tiles do a
            # plain tensor_tensor multiply and DVE row-sums the masked
            # attn afterwards (f16 SBUF -> 2x mode, ~0.6us vs 1.2us).
            sl = at_slice = None
            at = attn_t[si]
            sl = at[:, blk * 1024:(blk + 1) * 1024]
            if si in (POOL_A if blk < 2 else POOL_B):
                nc.gpsimd.tensor_tensor(sl, sl, masks[blk][si][:], op=ALU.mult)
                nc.vector.tensor_reduce(
                    den4[:, si, blk:blk + 1], sl, mybir.AxisListType.X,
                    ALU.add)
            else:
                nc.vector.scalar_tensor_tensor(
                    sl, sl, 1.0, masks[blk][si][:],
                    op0=ALU.bypass, op1=ALU.mult,
                    accum_out=den4[:, si, blk:blk + 1])

        with tc.tile_pool(name="xk", bufs=2) as xkp, \
                tc.tile_pool(name="xq", bufs=3) as xqp, \
                tc.tile_pool(name="psP", bufs=2, space="PSUM") as psP:
            xk_t, xq_t = [], []

            def alloc_xk(sj):
                xt = xkp.tile([128, n_d, 512], f16, name="xk_b")
                nc.sync.dma_start(xt[:], x_kP[sj])
                xk_t.append(xt)

            def alloc_xq(b):
                xt = xqp.tile([128, n_d, 512], f16, name="xq_b")
                nc.sync.dma_start(xt[:], x_qP[b])
                xq_t.append(xt)

            kproj_ps = {}

            def kproj_part(sj, part):
                # half the d-chunks per call so the PE insertion is small
                if part == 0:
                    kproj_ps[sj] = psP.tile([64, 512], f32, name="ps_k",
                                            tag="psp")
                ps = kproj_ps[sj]
                d0 = part * (n_d // 2)
                for di in range(d0, d0 + n_d // 2):
                    nc.tensor.matmul(ps[:], w_sb[:, 1, di, :],
                                     xk_t[sj][:, di, :], start=(di == 0),
                                     stop=(di == n_d - 1),
                                     skip_group_check=True)
                if part == 1:
                    nc.scalar.activation(
                        kT[:, sj * 512:(sj + 1) * 512], ps[:], AF.Identity,
                        bias=bk1)

            def qproj_pair(p):
                ps = psP.tile([64, 1024], f32, name="ps_q", tag="psp")
                for di in range(n_d):
                    w = w_sb[:, 0, di, :]
                    nc.tensor.matmul(
                        ps[:, 0:512], w, xq_t[2 * p][:, di, :],
                        start=(di == 0), stop=(di == n_d - 1),
                        skip_group_check=True)
                    nc.tensor.matmul(
                        ps[:, 512:1024], w, xq_t[2 * p + 1][:, di, :],
                        start=(di == 0), stop=(di == n_d - 1),
                        skip_group_check=True)
                # q = (raw + bq)/8 = raw*0.125 + bq/8
                nc.scalar.activation(
                    qT[:, p * 1024:(p + 1) * 1024], ps[:], AF.Identity,
                    bias=bq8, scale=0.125)

            def emit_score(si, p):
                ps = psA.tile([128, 1024], f32, name="ps_s", tag="ps")
                ks = kT[:, si * 128:(si + 1) * 128]
                nc.tensor.matmul(
                    ps[:, 0:512], ks, qT[:, (2 * p) * 512:(2 * p + 1) * 512],
                    start=True, stop=True, skip_group_check=True)
                nc.tensor.matmul(
                    ps[:, 512:1024], ks,
                    qT[:, (2 * p + 1) * 512:(2 * p + 2) * 512],
                    start=True, stop=True, skip_group_check=True)
                nc.scalar.activation(
                    attn_t[si][:, p * 1024:(p + 1) * 1024], ps[:], AF.Exp)

            # ---------------- preamble ----------------
            # dummy K=1 matmuls warm the PE HAM clock before the first
            # real projections arrive
            warm = psP.tile([128, 64], f32, name="warm", tag="psp")
            for i in range(40):
                nc.tensor.matmul(warm[:], ones_sb[:], bv_sb[:],
                                 start=(i == 0), stop=(i == 39),
                                 skip_group_check=True)
            nc.vector.tensor_scalar(
                den[:, 0:1], warm[:, 0:1], 0.0, None, op0=ALU.mult)
            alloc_xq(0)   # DMA priority order = consumption order
            alloc_xk(0)
            alloc_xq(1)
            alloc_xk(1)
            kproj_part(0, 0)
            kproj_part(0, 1)
            qproj_pair(0)
            alloc_mask(0, 0)
            alloc_mask(0, 1)
            alloc_xk(2)
            alloc_xk(3)
            alloc_xq(2)
            alloc_xq(3)

            # ---------------- sweep 1: q blocks 0,1 ----------------
            kp_sched = {1: (1, 0), 2: (1, 1), 3: (2, 0), 4: (2, 1),
                        5: (3, 0), 6: (3, 1)}
            for si in range(n_si):
                if si in kp_sched:
                    kproj_part(*kp_sched[si])
                at = attn_p.tile([128, sq], f16, name="attn_t")
                attn_t.append(at)
                emit_score(si, 0)
                if si + 2 < n_si:
                    alloc_mask(0, si + 2)
                if si >= 2:
                    stt_part(si - 2, 0)
                if si == 8:
                    alloc_xq(4)
                if si == 10:
                    qproj_pair(1)
                if si == 11:
                    alloc_xq(5)
                if si == 14:
                    alloc_mask(1, 0)
                if si == 15:
                    alloc_mask(1, 1)
            stt_part(n_si - 2, 0)
            stt_part(n_si - 1, 0)

            # ---------------- sweep 2: q blocks 2,3 ----------------
            for si in range(n_si):
                emit_score(si, 1)
                if si + 2 < n_si:
                    alloc_mask(1, si + 2)
                if si >= 2:
                    stt_part(si - 2, 1)
                if si == 0:
                    alloc_xq(6)
                if si == 6:
                    qproj_pair(2)
                if si == 8:
                    alloc_xq(7)
            stt_part(n_si - 2, 1)
            stt_part(n_si - 1, 1)
            qproj_pair(3)

        # ---------------- phase B: q blocks 4..7 + v + y ----------------
        xvp = ctx.enter_context(tc.tile_pool(name="xv", bufs=2))
        xv_q = []

        def alloc_xv(qi):
            xt = xvp.tile([128, n_d, n_vq * 128], f16, name="xv_t")
            nc.sync.dma_start(xt[:], x_vP[qi])
            xv_q.append(xt)

        def emit_v(si):
            ps = psA.tile([128, dk], f32, name="ps_v", tag="ps")
            xt = xv_q[si // n_vq]
            c0 = (si % n_vq) * 128
            for di in range(n_d):
                nc.tensor.matmul(
                    ps[:], xt[:, di, c0:c0 + 128], w_sb[:, 2, di, :],
                    start=(di == 0), stop=False)
            nc.tensor.matmul(ps[:], ones_sb[:], bv_sb[:], start=False,
                             stop=True)
            nc.vector.tensor_scalar(
                vs_sb[:, si, :], ps[:], rec[:, si:si + 1], None, op0=ALU.mult)

        psY = ctx.enter_context(tc.tile_pool(name="psY", bufs=1, space="PSUM"))
        yps = [psY.tile([128, 512], f32, name=f"yps{j}", tag=f"yps{j}")
               for j in range(n_qp)]

        def emit_y(si):
            for j in range(n_qp):
                nc.tensor.matmul(
                    yps[j][0:64, :], vs_sb[:, si, :],
                    attn_t[si][:, j * 512:(j + 1) * 512],
                    start=(si == 0), stop=(si == n_si - 1),
                    skip_group_check=True)
                nc.tensor.matmul(
                    yps[j][64:128, :], vs_sb[:, si, :],
                    attn_t[si][:, (j + n_qp) * 512:(j + n_qp + 1) * 512],
                    start=(si == 0), stop=(si == n_si - 1),
                    skip_group_check=True)

        alloc_mask(2, 0)
        alloc_mask(2, 1)
        alloc_xv(0)
        alloc_mask(3, 0)
        alloc_mask(3, 1)
        alloc_xv(1)

        for si in range(n_si):
            if si + 2 < n_si:
                alloc_mask(2, si + 2)
                alloc_mask(3, si + 2)
            if si == 6:
                alloc_xv(2)
            if si == 10:
                alloc_xv(3)
            emit_score(si, 2)
            emit_score(si, 3)
            stt_part(si, 2)
            stt_part(si, 3)
            nc.vector.tensor_reduce(
                den[:, si:si + 1], den4[:, si, :], mybir.AxisListType.X,
                ALU.add)
            nc.vector.reciprocal(rec[:, si:si + 1], den[:, si:si + 1])
            if si >= V_LAG:
                emit_v(si - V_LAG)
            if si >= Y_LAG:
                emit_y(si - Y_LAG)
        for si in range(max(n_si - V_LAG, 0), n_si):
            emit_v(si)
        for si in range(max(n_si - Y_LAG, 0), n_si):
            emit_y(si)

        # ---------------- output ----------------
        y_p = ctx.enter_context(tc.tile_pool(name="yp", bufs=1))
        y_sb = y_p.tile([128, sq // 2], f16, name="y_sb")
        for j in range(n_qp):
            dst = y_sb[:, j * 512:(j + 1) * 512]
            if j % 2 == 0:
                nc.scalar.activation(dst, yps[j][:], AF.Copy)
            else:
                nc.vector.tensor_scalar(dst, yps[j][:], 1.0, None,
                                        op0=ALU.mult)
        nc.sync.dma_start(out[:], y_sb[:])


def build_nc(sq=SQ, sk=SK_SHARD, d=D_MODEL, dk=D_K):
    """Build + compile the per-core Bacc module."""
    import concourse.tile as tile
    from concourse import bacc, mybir

    f16 = mybir.dt.float16
    f32 = mybir.dt.float32
    u8 = mybir.dt.uint8
    n_d = d // 128

    nc = bacc.Bacc("TRN2", target_bir_lowering=False, debug=False)
    n_vq = max((sk // 128) // 4, 1)
    aps = {
        "x_qP": nc.dram_tensor("x_qP", [sq // 512, 128, n_d, 512], f16,
                               kind="ExternalInput").ap(),
        "x_kP": nc.dram_tensor("x_kP", [sk // 512, 128, n_d, 512], f16,
                               kind="ExternalInput").ap(),
        "x_vP": nc.dram_tensor("x_vP", [4, 128, n_d, n_vq * 128], f16,
                               kind="ExternalInput").ap(),
        "maskT": nc.dram_tensor("maskT", [sk, sq], u8, kind="ExternalInput").ap(),
        "wall": nc.dram_tensor("wall", [128, 3, n_d, dk], f16,
                               kind="ExternalInput").ap(),
        "bias": nc.dram_tensor("bias", [128, 2 + dk], f32,
                               kind="ExternalInput").ap(),
        "bv16": nc.dram_tensor("bv16", [1, dk], f16, kind="ExternalInput").ap(),
        "out": nc.dram_tensor("out", [128, sq // 2], f16,
                              kind="ExternalOutput").ap(),
    }
    with tile.TileContext(nc) as tc:
        emit_kernel(tc, aps, sq, sk, d, dk)
    nc.compile()
    return nc


def pack_cols(xT, block):
    """[d, n] -> [n/block, 128, d/128, block] contiguous: per column-block, the
    exact SBUF tile image ([partition, d-chunk, col])."""
    d, n = xT.shape
    return np.ascontiguousarray(
        xT.reshape(d // 128, 128, n // block, block).transpose(2, 1, 0, 3))


def make_in_maps(x_q, x_k, x_v, mask, Wq, bq, Wk, bk, Wv, bv, sk_shard=SK_SHARD):
    """Host-side sharding + layout prep. Returns list of per-core input dicts."""
    f16 = np.float16
    d, dk = Wq.shape[1], Wq.shape[0]
    n_d = d // 128
    n_shards = x_k.shape[1] // sk_shard

    wall = np.empty((128, 3, n_d, dk), f16)
    for i, W in enumerate((Wq, Wk, Wv)):
        WT = W.T.astype(f16)  # [d, dk]
        for di in range(n_d):
            wall[:, i, di, :] = WT[di * 128:(di + 1) * 128, :]
    bias = np.empty((128, 2 + dk), np.float32)
    bias[:, 0] = np.tile(np.asarray(bq, np.float32) / 8.0, 128 // dk)
    bias[:, 1] = np.tile(np.asarray(bk, np.float32), 128 // dk)
    bias[:, 2:] = np.asarray(bv, np.float32)[None, :]
    bv16 = np.asarray(bv, np.float32).astype(f16).reshape(1, dk)

    n_vq = max((sk_shard // 128) // 4, 1)
    xqP = [pack_cols(x_q[b].T.astype(f16), 512) for b in range(x_q.shape[0])]
    in_maps = []
    for b in range(x_q.shape[0]):
        for h in range(n_shards):
            sl = slice(h * sk_shard, (h + 1) * sk_shard)
            in_maps.append({
                "x_qP": xqP[b],
                "x_kP": pack_cols(x_k[b, sl, :].T.astype(f16), 512),
                "x_vP": pack_cols(x_v[b, sl, :].T.astype(f16), n_vq * 128),
                "maskT": np.ascontiguousarray(mask[b, :, sl].T).astype(np.uint8),
                "wall": wall, "bias": bias, "bv16": bv16,
            })
    return in_maps


def unpack_out(o, sq=SQ, dk=D_K):
    """out [128, sq/2] f16 -> yT [dk, sq] f32. Top half: q-chunks 0..nq/2-1,
    bottom half: q-chunks nq/2..nq-1."""
    yT = np.empty((dk, sq), np.float32)
    half = sq // 2
    yT[:, 0:half] = o[0:dk, :].astype(np.float32)
    yT[:, half:sq] = o[64:64 + dk, :].astype(np.float32)
    return yT


_NC_CACHE = {}
# test.py can set extra run_bass_kernel_spmd kwargs here (e.g. trace=True)
RUN_KWARGS = {}


def _get_nc():
    if "nc" not in _NC_CACHE:
        _NC_CACHE["nc"] = build_nc()
    return _NC_CACHE["nc"]


def kernel(**inputs):
    from concourse.bass_utils import run_bass_kernel_spmd

    x_q = np.asarray(inputs["x_q"], np.float32)
    x_k = np.asarray(inputs["x_k"], np.float32)
    x_v = np.asarray(inputs["x_v"], np.float32)
    mask = np.asarray(inputs["mask"])
    Wq, bq = np.asarray(inputs["Wq"], np.float32), np.asarray(inputs["bq"], np.float32)
    Wk, bk = np.asarray(inputs["Wk"], np.float32), np.asarray(inputs["bk"], np.float32)
    Wv, bv = np.asarray(inputs["Wv"], np.float32), np.asarray(inputs["bv"], np.float32)

    nc = _get_nc()
    in_maps = make_in_maps(x_q, x_k, x_v, mask, Wq, bq, Wk, bk, Wv, bv)
    res = run_bass_kernel_spmd(nc, in_maps, list(range(N_CORES)), **RUN_KWARGS)
    _NC_CACHE["last_res"] = res
    n_shards = N_CORES // x_q.shape[0]
    y = np.zeros((x_q.shape[0], SQ, D_K), np.float32)
    for core in range(N_CORES):
        y[core // n_shards] += unpack_out(res.results[core]["out"]).T
    return y
